# revision 29
# baseline (speedup 1.0000x reference)
"""Trainium2 Bass kernel for GroupNorm + single-head self-attention block.

Computes, per batch element b (data-parallel over 8 NeuronCores):
    xn = group_norm(x[b])                 # 8 groups over (H, W, C/8)
    q, k, v = xn@wq+bq, xn@wk+bk, xn@wv+bv
    attn = softmax(q @ k.T / sqrt(C))
    y[b] = xn + (attn @ v) @ wp + bp

Shapes: x [8, 64, 64, 128] -> per core [4096, 128], C=128.

v2 dataflow (per core), all-bf16 compute path:
  - x DMA'd in 4 slabs; PE transposes -> xT f32; DVE copies accumulate
    group-norm stats per 8-tile group (sum via reduce, sumsq via
    scalar_tensor_tensor with accum_out).
  - xnT = a*xT + b emitted as bf16 (used for q/k/v and the residual).
  - qT/kT bf16 with bias folded into the DVE psum->sbuf copy; v tiles
    bf16 with a ones column; bv/bp folded into c_p = bv@wp + bp added
    via a rank-1 matmul in the output projection.
  - main loop is software-pipelined across chunk boundaries: score
    matmuls (and exp on ScalarE) run 2 steps ahead of the attn@V
    accumulation, so the PE stream never waits on the chunk tails.
  - out accumulation keeps the baseline trick: column 128 of each
    129-wide slot accumulates the softmax denominator for free.
  - chunk tail: normalize (DVE), PE transpose (bf16), project with wp +
    residual (identity matmul on xnT) + c_p rank-1, DVE copy, DMA out.
"""

import numpy as np

import concourse.bass as bass
import concourse.bacc as bacc
import concourse.mybir as mybir
import concourse.tile as tile
from concourse.bass_utils import run_bass_kernel_spmd
from concourse.tile import add_dep_helper

F32 = mybir.dt.float32
F32R = mybir.dt.float32r
BF16 = mybir.dt.bfloat16
F8 = mybir.dt.float8e4
U8 = mybir.dt.uint8
# fp8-e4m3 exp bits: bits = sat_u8(s*8*log2e + B); shift of -3.5 and the
# Schraudolph offset folded into B; negatives saturate to 0 (= fp8 +0)
EXP_A = 8 * 1.4426950408889634
EXP_B = -3.5 * 8 * 1.4426950408889634 + 56 - 0.25
AF = mybir.ActivationFunctionType
ALU = mybir.AluOpType
AX = mybir.AxisListType

B, H, W, C = 8, 64, 64, 128
NQ = H * W  # 4096 tokens per batch element
GROUPS = 8
EPS = 1e-5
N_CORES = 8

LAST_RESULTS = None  # BassKernelResults of the most recent run (for profiling)


def _body(tc, d, nq, stage=99):
    nc = tc.nc
    nj = nq // 128              # k-tiles (32)
    chq = min(512, nq)          # q-chunk width
    nch = nq // chq             # chunks (8)
    qsn = chq // 128            # q-subtiles per chunk (4)
    assert qsn == 4 and nj % 2 == 0, (nq, qsn)
    npair = nj // 2             # jp pairs per chunk (16)
    scale = float(C) ** -0.5

    cp = tc.alloc_tile_pool(name="consts", bufs=1)
    big = tc.alloc_tile_pool(name="big", bufs=1)
    misc = tc.alloc_tile_pool(name="misc", bufs=2, space="PSUM")
    pro = tc.alloc_tile_pool(name="pro", bufs=6, space="PSUM")
    p_sc = p_out = None  # created after pro.release() (PSUM is bank-budgeted)
    sb_p = tc.alloc_tile_pool(name="sb_p", bufs=5)
    sb_t = tc.alloc_tile_pool(name="sb_t", bufs=2)

    # ---------------- x DMA first (transposes gate the whole prologue) ----
    xsb = big.tile([128, nj, 128], F32)
    x_r = d["x"].ap().rearrange("(s t p) c -> s p t c", p=128, t=4)
    nc.sync.dma_start(xsb[:, 0:4, :], x_r[0])
    ident = cp.tile([C, C], F32)
    nc.sync.dma_start(ident[:, :], d["ident"].ap())
    for s in range(1, 8):
        nc.sync.dma_start(xsb[:, s * 4:(s + 1) * 4, :], x_r[s])
    # warm the Ln/Exp activation tables while the DMAs stream
    warm = cp.tile([1, 2], F32)
    nc.vector.memset(warm[:, :], 1.0)
    nc.scalar.activation(warm[:, 0:1], warm[:, 0:1], AF.Ln, bias=warm[:, 1:2])
    nc.scalar.activation(warm[:, 0:1], warm[:, 0:1], AF.Exp)
    ident_b = cp.tile([C, C], BF16)
    nc.vector.tensor_copy(ident_b[:, :], ident[:, :])
    gmat = cp.tile([C, GROUPS], F32)
    nc.sync.dma_start(gmat[:, :], d["gmat"].ap())
    gtmat = cp.tile([GROUPS, C], F32)
    nc.sync.dma_start(gtmat[:, :], d["gtmat"].ap())
    ones_row = cp.tile([1, C], BF16)
    nc.vector.memset(ones_row[:, :], 1.0)
    negc = cp.tile([128, 1], F32)
    nc.vector.memset(negc[:, :], -3.5)

    wsb = {}
    for wname in ("wq", "wk", "wv", "wp"):
        wf = cp.tile([C, C], F32, name=f"{wname}_f")
        nc.sync.dma_start(wf[:, :], d[wname].ap())
        wsb[wname] = cp.tile([C, C], BF16, name=f"{wname}_sb")
        if wname in ("wq", "wk"):  # fold sqrt of attention scale into each
            nc.vector.tensor_scalar_mul(wsb[wname][:, :], wf[:, :],
                                        scale ** 0.5)
        else:
            nc.vector.tensor_copy(wsb[wname][:, :], wf[:, :])
    # bias columns [C,1] f32 for per-partition folds (q/k); bv as bf16 col
    bcol = {}
    for bname in ("bq", "bk", "bv", "bp"):
        bf = cp.tile([1, C], F32, name=f"{bname}_f")
        nc.sync.dma_start(bf[:, :], d[bname].ap().rearrange("(o c) -> o c", o=1))
        bcol[bname] = bf
    gamma_c = cp.tile([C, 1], F32)
    nc.sync.dma_start(gamma_c[:, :], d["gamma"].ap().rearrange("(c o) -> c o", o=1))
    beta_c = cp.tile([C, 1], F32)
    nc.sync.dma_start(beta_c[:, :], d["beta"].ap().rearrange("(c o) -> c o", o=1))

    # ---------------- x transpose to xT + stats ----------------
    # transposes use the deep pro psum ring; psum->sbuf copies alternate
    # DVE / ScalarE so neither engine gates the prologue; per-8-tile-group
    # stats (sum via reduce, sumsq via stt+accum) trail the copies on DVE.
    xT = big.tile([C, nq], BF16)
    ngr = 8
    s1 = cp.tile([C, ngr], F32)
    s2 = cp.tile([C, ngr], F32)
    xsq = big.tile([C, nq // ngr], BF16)  # overwritten per group; accum matters
    gw = nq // ngr
    for t in range(nj):
        pst = pro.tile([128, 128], F32, name="xtp", tag="pro")
        nc.tensor.transpose(pst[:, :], xsb[:, t, :], ident[:, :])
        sl = slice(t * 128, (t + 1) * 128)
        if t % 4 == 0:
            nc.vector.tensor_copy(xT[:, sl], pst[:, :])
        else:
            nc.scalar.copy(xT[:, sl], pst[:, :])
        if t % 4 == 3:
            g = t // 4
            gsl = slice(g * gw, (g + 1) * gw)
            nc.vector.reduce_sum(s1[:, g:g + 1], xT[:, gsl], axis=AX.X)
            nc.vector.scalar_tensor_tensor(
                xsq[:, :], xT[:, gsl], 1.0, xT[:, gsl],
                op0=ALU.mult, op1=ALU.mult, accum_out=s2[:, g:g + 1])

    pro.release()
    kpool = tc.alloc_tile_pool(name="kpool", bufs=4, space="PSUM")

    # bias transposes + c_p (emitted after the x transposes so the PE
    # stream is not gated on the late bias DMAs)
    bq_c = cp.tile([C, 1], F32)
    bk_c = cp.tile([C, 1], F32)
    bv_c = cp.tile([C, 1], BF16)
    bqs_row = cp.tile([1, C], F32)
    nc.vector.tensor_scalar_mul(bqs_row[:, :], bcol["bq"][:, :], scale ** 0.5)
    bks_row = cp.tile([1, C], F32)
    nc.vector.tensor_scalar_mul(bks_row[:, :], bcol["bk"][:, :], scale ** 0.5)
    for row, dst in ((bqs_row, bq_c), (bks_row, bk_c), (bcol["bv"], bv_c)):
        ps_b = misc.tile([C, 512], F32, name="ps_b", tag="misc")
        nc.tensor.transpose(ps_b[:, 0:1], row[:, :], ident[0:1, 0:1])
        nc.vector.tensor_copy(dst[:, :], ps_b[:, 0:1])

    # c_p = bv @ wp + bp  (rank-1 fold of the v/p biases)
    cp_row = cp.tile([1, C], BF16)
    ps_cp = misc.tile([C, 512], F32, name="ps_cp", tag="misc")
    nc.tensor.matmul(ps_cp[0:1, 0:C], bv_c[:, :], wsb["wp"][:, :],
                     start=True, stop=True)
    nc.vector.tensor_tensor(cp_row[:, :], ps_cp[0:1, 0:C], bcol["bp"][:, :],
                            op=ALU.add)

    def _flat_out(src_ap):
        yf = d["y"].ap().rearrange("n c -> (n c)").rearrange(
            "(p f) -> p f", p=128)
        nc.sync.dma_start(yf, src_ap)

    if stage == 1:
        xt_f = big.tile([C, nq], F32)
        nc.vector.tensor_copy(xt_f[:, :], xT[:, :])
        _flat_out(xt_f[:, :])
        for p in (sb_t, sb_p, pro, misc, big, cp):
            p.release()
        return

    st2 = cp.tile([C, 2], F32)
    nc.vector.reduce_sum(st2[:, 0:1], s1[:, :], axis=AX.X)
    nc.vector.reduce_sum(st2[:, 1:2], s2[:, :], axis=AX.X)
    gps = misc.tile([GROUPS, 512], F32, name="gps", tag="misc")
    # gmat is pre-scaled by 1/(nq*C/G): gps = [mean | E[x^2]] directly
    nc.tensor.matmul(gps[:, 0:2], gmat[:, :], st2[:, :], start=True, stop=True)
    gstat = cp.tile([GROUPS, 6], F32)
    nc.vector.tensor_copy(gstat[:, 0:2], gps[:, 0:2])
    nc.vector.tensor_mul(gstat[:, 2:3], gstat[:, 0:1], gstat[:, 0:1])     # mean^2
    nc.vector.tensor_sub(gstat[:, 3:4], gstat[:, 1:2], gstat[:, 2:3])     # var
    # rstd = exp(-0.5*ln(var+eps)) — ln/exp live in one ACT table set
    eps_c = cp.tile([GROUPS, 1], F32)
    nc.vector.memset(eps_c[:, :], EPS)
    nc.scalar.activation(gstat[:, 4:5], gstat[:, 3:4], AF.Ln, bias=eps_c[:, :])
    nc.scalar.activation(gstat[:, 5:6], gstat[:, 4:5], AF.Exp, scale=-0.5)
    pair = cp.tile([GROUPS, 2], F32)
    nc.vector.tensor_copy(pair[:, 0:1], gstat[:, 5:6])
    nc.vector.tensor_copy(pair[:, 1:2], gstat[:, 0:1])
    bcp = misc.tile([C, 512], F32, name="bcp", tag="misc")
    nc.tensor.matmul(bcp[:, 0:2], gtmat[:, :], pair[:, :], start=True, stop=True)
    ab = cp.tile([C, 2], F32)
    nc.vector.tensor_mul(ab[:, 0:1], gamma_c[:, :], bcp[:, 0:1])          # a
    nc.vector.tensor_mul(ab[:, 1:2], bcp[:, 1:2], ab[:, 0:1])             # mean*a
    nc.vector.tensor_sub(ab[:, 1:2], beta_c[:, :], ab[:, 1:2])            # b

    # xnT in 512-col pieces, each chased by its kT projection (PE + DVE);
    # only piece 0 runs in the prologue, the rest stream into the main loop
    xnT = big.tile([C, nq], BF16)
    kT = big.tile([C, nq], F8)

    def emit_kT(ch):
        sl = slice(ch * 512, (ch + 1) * 512)
        ps = kpool.tile([128, 512], F32, name="k_ps", tag="kpool")
        nc.tensor.matmul(ps[:, :], wsb["wk"][:, :], xnT[:, sl],
                         start=True, stop=True)
        if ch % 4 != 3:
            nc.scalar.activation(kT[:, sl], ps[:, :], AF.Identity,
                                 bias=bk_c[:, :])
        else:
            nc.vector.tensor_scalar_add(kT[:, sl], ps[:, :], bk_c[:, :])

    for ch in range(nch):
        sl = slice(ch * 512, (ch + 1) * 512)
        nc.vector.tensor_scalar(
            xnT[:, sl], xT[:, sl], ab[:, 0:1], ab[:, 1:2],
            op0=ALU.mult, op1=ALU.add)
    for ch in range(nch):
        emit_kT(ch)

    if stage == 2:
        xn_f = big.tile([C, nq], F32)
        nc.vector.tensor_copy(xn_f[:, :], xnT[:, :])
        _flat_out(xn_f[:, :])
        for p in (sb_t, sb_p, pro, misc, big, cp):
            p.release()
        return

    # ---------------- projections: v1 (deferred), qT (per chunk) --------
    v1 = big.tile([128, nj, 130], F8)
    nc.vector.memset(v1[:, :, 128:130], 1.0)

    def emit_v1(t, pool=None):
        ps = (pool or misc).tile([128, 512], F32, name="v_ps",
                                 tag="misc" if pool is None else "kpool")
        nc.tensor.matmul(ps[:, 0:128], xnT[:, t * 128:(t + 1) * 128],
                         wsb["wv"][:, :], start=True, stop=True)
        nc.vector.tensor_copy(v1[:, t, 0:128], ps[:, 0:128])

    qT = big.tile([C, nq], F8)

    def emit_qT(ch, pool=None):
        sl = slice(ch * 512, (ch + 1) * 512)
        ps = (pool or misc).tile([128, 512], F32, name="q_ps",
                                 tag="misc" if pool is None else "kpool")
        nc.tensor.matmul(ps[:, :], wsb["wq"][:, :], xnT[:, sl],
                         start=True, stop=True)
        if ch < 2:
            nc.scalar.activation(qT[:, sl], ps[:, :], AF.Identity,
                                 bias=bq_c[:, :])
        else:
            nc.vector.tensor_scalar_add(qT[:, sl], ps[:, :], bq_c[:, :])

    emit_qT(0, pool=kpool)
    emit_qT(1, pool=kpool)
    kpool.release()
    p_sc = tc.alloc_tile_pool(name="p_sc", bufs=2, space="PSUM")
    p_out = tc.alloc_tile_pool(name="p_out", bufs=1, space="PSUM")

    if stage == 3:
        for t in range(nj):
            emit_v1(t)
        kt_f = big.tile([C, nq], F32)
        nc.vector.tensor_copy(kt_f[:, :], kT[:, :])
        _flat_out(kt_f[:, :])
        for p in (sb_t, sb_p, p_out, p_sc, misc, big, cp):
            p.release()
        return

    # ---------------- main attention loop (software-pipelined) ----------
    y_r = d["y"].ap().rearrange("(c q p) ch -> c p q ch", q=qsn, p=128)
    LA = 4  # scores/exp run LA steps ahead of attnV

    state = {}  # per-chunk in-flight psum/sbuf handles

    def emit_scores(step):
        ch, jp = divmod(step, npair)
        sc = p_sc.tile([128, 2, 512], F32, name="sc")
        for jj in range(2):
            j = 2 * jp + jj
            nc.tensor.matmul(sc[:, jj, 0:chq],
                             kT[:, j * 128:(j + 1) * 128],
                             qT[:, ch * chq:(ch + 1) * chq],
                             start=True, stop=True)
        pT = sb_p.tile([128, 2, 512], F8, name="pT")
        # exp on ScalarE, shifted by -3.5 to fit fp8-e4m3 range (softmax
        # is shift-invariant)
        nc.scalar.activation(pT[:, :, 0:chq], sc[:, :, 0:chq], AF.Exp,
                             bias=negc[:, :])
        state[(ch, jp)] = pT

    def emit_attnv(step):
        ch, jp = divmod(step, npair)
        if jp == 0:
            out_ac = p_out.tile([128, 2, 512], F32, name="out_ac")
            state[ch] = (out_ac, {}, {})
        out_ac, first_mm, last_mm = state[ch]
        pT = state.pop((ch, jp))
        for b_ in range(2):
            for s in range(2):
                qs = 2 * b_ + s
                mm = nc.tensor.matmul(
                    out_ac[:, b_, 129 * s:129 * s + 129],
                    pT[:, 0:2, qs * 128:(qs + 1) * 128],
                    v1[:, 2 * jp:2 * jp + 2, 0:129],
                    start=(jp == 0 and s == 0),
                    stop=(jp == npair - 1 and s == 1),
                    perf_mode=mybir.MatmulPerfMode.DoubleRow)
                first_mm.setdefault((b_, s), mm)
                last_mm[(b_, s)] = mm
        if jp == npair - 1:
            for b_ in range(2):
                add_dep_helper(first_mm[(b_, 1)].ins, first_mm[(b_, 0)].ins,
                               sync=False, reason="psum group start order")
                add_dep_helper(last_mm[(b_, 1)].ins, last_mm[(b_, 0)].ins,
                               sync=False, reason="psum group stop order")

    tails = {}

    def emit_tail_dve(ch):
        out_ac, _, _ = state.pop(ch)
        rcp = sb_t.tile([128, 2, 2, 1], F32, name="rcp")
        den = out_ac[:, :, 128:128 + 258].rearrange(
            "p b (s x) -> p b s x", s=2, x=129)[:, :, :, 0:1]
        nc.vector.reciprocal(rcp[:, :, :, :], den)
        attn = sb_t.tile([128, qsn, 128], BF16, name="attn")
        ysb = sb_t.tile([128, qsn, 128], F32, name="ysb")
        for qs in range(qsn):
            b_, s = qs // 2, qs % 2
            nc.vector.tensor_scalar_mul(
                attn[:, qs, :], out_ac[:, b_, 129 * s:129 * s + 128],
                rcp[:, b_, s, :])
        if stage == 4:
            ysb4 = sb_t.tile([128, qsn, 128], F32, name="ysb4")
            nc.vector.tensor_copy(ysb4[:, :, :], attn[:, :, :])
            nc.sync.dma_start(y_r[ch], ysb4[:, :, :])
            return
        tails[ch] = (attn, ysb, {})

    def emit_tail_piece(ch, k):
        # piece k: transpose subtile k (k<qsn); project subtile k-1 (k>0)
        attn, ysb, atTs = tails[ch]
        if k < qsn:
            pst = misc.tile([128, 512], BF16, name="pst", tag="misc")
            nc.tensor.transpose(pst[:, 0:128], attn[:, k, :], ident_b[:, :])
            atT = sb_t.tile([128, 128], BF16, name="atT")
            nc.vector.tensor_copy(atT[:, :], pst[:, 0:128])
            atTs[k] = atT
        if k > 0:
            qs = k - 1
            psf = misc.tile([128, 512], F32, name="psf", tag="misc")
            nc.tensor.matmul(psf[:, 0:128], atTs.pop(qs)[:, :],
                             wsb["wp"][:, :], start=True, stop=False)
            nt = ch * chq + qs * 128
            nc.tensor.matmul(psf[:, 0:128], xnT[:, nt:nt + 128],
                             ident_b[:, :], start=False, stop=False)
            nc.tensor.matmul(psf[:, 0:128], ones_row[:, 0:128],
                             cp_row[:, :], start=False, stop=True)
            nc.vector.tensor_copy(ysb[:, qs, :], psf[:, 0:128])
            if qs == qsn - 1:
                nc.sync.dma_start(y_r[ch], ysb[:, :, :])
                del tails[ch]

    nsteps = nch * npair
    pend = {}
    for step in range(nsteps + LA + 2 * qsn + 3):
        if step < nsteps:
            ch_s, jp_s = divmod(step, npair)
            if jp_s == npair // 2 and ch_s + 2 < nch:
                emit_qT(ch_s + 2)
            emit_scores(step)
        if 2 * step < nj:
            emit_v1(2 * step)
            emit_v1(2 * step + 1)
        if LA <= step < nsteps + LA:
            a = step - LA
            emit_attnv(a)
            ch_a, jp_a = divmod(a, npair)
            if jp_a == npair - 1:
                emit_tail_dve(ch_a)
                if stage != 4:
                    for k in range(qsn + 1):
                        pend.setdefault(step + 1 + 2 * k, []).append((ch_a, k))
        for (c_, k_) in pend.pop(step, []):
            emit_tail_piece(c_, k_)

    for p in (sb_t, sb_p, p_out, p_sc, misc, big, cp):
        p.release()


def build_module(nq=NQ, stage=99):
    nc = bacc.Bacc("TRN2", target_bir_lowering=False, debug=False,
                   enable_asserts=False)
    d = {}
    d["x"] = nc.dram_tensor("x", [nq, C], F32, kind="ExternalInput")
    d["gamma"] = nc.dram_tensor("gamma", [C], F32, kind="ExternalInput")
    d["beta"] = nc.dram_tensor("beta", [C], F32, kind="ExternalInput")
    for wname in ("wq", "wk", "wv", "wp"):
        d[wname] = nc.dram_tensor(wname, [C, C], F32, kind="ExternalInput")
    for bname in ("bq", "bk", "bv", "bp"):
        d[bname] = nc.dram_tensor(bname, [C], F32, kind="ExternalInput")
    d["y"] = nc.dram_tensor("y", [nq, C], F32, kind="ExternalOutput")

    d["ident"] = nc.inline_tensor(np.eye(C, dtype=np.float32), "ident")
    gm = np.zeros((C, GROUPS), np.float32)
    gm[np.arange(C), np.arange(C) // (C // GROUPS)] = 1.0
    d["gtmat"] = nc.inline_tensor(np.ascontiguousarray(gm.T), "gtmat")
    d["gmat"] = nc.inline_tensor(gm / (nq * (C // GROUPS)), "gmat")

    with tile.TileContext(nc) as tc:
        _body(tc, d, nq, stage=stage)
    nc.compile()
    return nc


_CACHED_NC = None


def kernel(x, gamma, beta, wq, bq, wk, bk, wv, bv, wp, bp):
    global _CACHED_NC, LAST_RESULTS
    x = np.asarray(x, np.float32)
    assert x.shape == (B, H, W, C), x.shape
    if _CACHED_NC is None:
        _CACHED_NC = build_module(NQ)
    nc = _CACHED_NC

    shared = {
        "gamma": np.asarray(gamma, np.float32),
        "beta": np.asarray(beta, np.float32),
        "wq": np.asarray(wq, np.float32), "bq": np.asarray(bq, np.float32),
        "wk": np.asarray(wk, np.float32), "bk": np.asarray(bk, np.float32),
        "wv": np.asarray(wv, np.float32), "bv": np.asarray(bv, np.float32),
        "wp": np.asarray(wp, np.float32), "bp": np.asarray(bp, np.float32),
    }
    xf = x.reshape(B, NQ, C)
    in_maps = [dict(shared, x=np.ascontiguousarray(xf[b_])) for b_ in range(B)]
    res = run_bass_kernel_spmd(nc, in_maps, core_ids=list(range(N_CORES)))
    LAST_RESULTS = res
    out = np.stack([res.results[b_]["y"] for b_ in range(B)])
    return out.reshape(B, H, W, C).astype(np.float32)


# revision 30
# speedup vs baseline: 1.0202x; 1.0202x over previous
"""Trainium2 Bass kernel for GroupNorm + single-head self-attention block.

Computes, per batch element b (data-parallel over 8 NeuronCores):
    xn = group_norm(x[b])                 # 8 groups over (H, W, C/8)
    q, k, v = xn@wq+bq, xn@wk+bk, xn@wv+bv
    attn = softmax(q @ k.T / sqrt(C))
    y[b] = xn + (attn @ v) @ wp + bp

Shapes: x [8, 64, 64, 128] -> per core [4096, 128], C=128.

v2 dataflow (per core), all-bf16 compute path:
  - x DMA'd in 4 slabs; PE transposes -> xT f32; DVE copies accumulate
    group-norm stats per 8-tile group (sum via reduce, sumsq via
    scalar_tensor_tensor with accum_out).
  - xnT = a*xT + b emitted as bf16 (used for q/k/v and the residual).
  - qT/kT bf16 with bias folded into the DVE psum->sbuf copy; v tiles
    bf16 with a ones column; bv/bp folded into c_p = bv@wp + bp added
    via a rank-1 matmul in the output projection.
  - main loop is software-pipelined across chunk boundaries: score
    matmuls (and exp on ScalarE) run 2 steps ahead of the attn@V
    accumulation, so the PE stream never waits on the chunk tails.
  - out accumulation keeps the baseline trick: column 128 of each
    129-wide slot accumulates the softmax denominator for free.
  - chunk tail: normalize (DVE), PE transpose (bf16), project with wp +
    residual (identity matmul on xnT) + c_p rank-1, DVE copy, DMA out.
"""

import numpy as np

import concourse.bass as bass
import concourse.bacc as bacc
import concourse.mybir as mybir
import concourse.tile as tile
from concourse.bass_utils import run_bass_kernel_spmd
from concourse.tile import add_dep_helper

F32 = mybir.dt.float32
F32R = mybir.dt.float32r
BF16 = mybir.dt.bfloat16
F8 = mybir.dt.float8e4
U8 = mybir.dt.uint8
# fp8-e4m3 exp bits: bits = sat_u8(s*8*log2e + B); shift of -3.5 and the
# Schraudolph offset folded into B; negatives saturate to 0 (= fp8 +0)
EXP_A = 8 * 1.4426950408889634
EXP_B = -3.5 * 8 * 1.4426950408889634 + 56 - 0.25
AF = mybir.ActivationFunctionType
ALU = mybir.AluOpType
AX = mybir.AxisListType

B, H, W, C = 8, 64, 64, 128
NQ = H * W  # 4096 tokens per batch element
GROUPS = 8
EPS = 1e-5
N_CORES = 8

LAST_RESULTS = None  # BassKernelResults of the most recent run (for profiling)


def _body(tc, d, nq, stage=99):
    nc = tc.nc
    nj = nq // 128              # k-tiles (32)
    chq = min(512, nq)          # q-chunk width
    nch = nq // chq             # chunks (8)
    qsn = chq // 128            # q-subtiles per chunk (4)
    assert qsn == 4 and nj % 2 == 0, (nq, qsn)
    npair = nj // 2             # jp pairs per chunk (16)
    scale = float(C) ** -0.5

    cp = tc.alloc_tile_pool(name="consts", bufs=1)
    big = tc.alloc_tile_pool(name="big", bufs=1)
    misc = tc.alloc_tile_pool(name="misc", bufs=2, space="PSUM")
    pro = tc.alloc_tile_pool(name="pro", bufs=6, space="PSUM")
    p_sc = p_out = None  # created after pro.release() (PSUM is bank-budgeted)
    sb_p = tc.alloc_tile_pool(name="sb_p", bufs=5)
    sb_t = tc.alloc_tile_pool(name="sb_t", bufs=2)

    # ---------------- x DMA first (transposes gate the whole prologue) ----
    xsb = big.tile([128, nj, 128], F32)
    x_r = d["x"].ap().rearrange("(s t p) c -> s p t c", p=128, t=4)
    nc.sync.dma_start(xsb[:, 0:4, :], x_r[0])
    ident = cp.tile([C, C], F32)
    nc.sync.dma_start(ident[:, :], d["ident"].ap())
    for s in range(1, 8):
        nc.sync.dma_start(xsb[:, s * 4:(s + 1) * 4, :], x_r[s])
    # warm the Ln/Exp activation tables while the DMAs stream
    warm = cp.tile([1, 2], F32)
    nc.vector.memset(warm[:, :], 1.0)
    nc.scalar.activation(warm[:, 0:1], warm[:, 0:1], AF.Ln, bias=warm[:, 1:2])
    nc.scalar.activation(warm[:, 0:1], warm[:, 0:1], AF.Exp)
    ident_b = cp.tile([C, C], BF16)
    nc.vector.tensor_copy(ident_b[:, :], ident[:, :])
    gmat = cp.tile([C, GROUPS], F32)
    nc.sync.dma_start(gmat[:, :], d["gmat"].ap())
    gtmat = cp.tile([GROUPS, C], F32)
    nc.sync.dma_start(gtmat[:, :], d["gtmat"].ap())
    ones_row = cp.tile([1, C], BF16)
    nc.vector.memset(ones_row[:, :], 1.0)
    negc = cp.tile([128, 1], F32)
    nc.vector.memset(negc[:, :], -3.5)

    wsb = {}
    for wname in ("wq", "wk", "wv", "wp"):
        wf = cp.tile([C, C], F32, name=f"{wname}_f")
        nc.sync.dma_start(wf[:, :], d[wname].ap())
        wsb[wname] = cp.tile([C, C], BF16, name=f"{wname}_sb")
        if wname in ("wq", "wk"):  # fold sqrt of attention scale into each
            nc.vector.tensor_scalar_mul(wsb[wname][:, :], wf[:, :],
                                        scale ** 0.5)
        else:
            nc.vector.tensor_copy(wsb[wname][:, :], wf[:, :])
    # bias columns [C,1] f32 for per-partition folds (q/k); bv as bf16 col
    bcol = {}
    for bname in ("bq", "bk", "bv", "bp"):
        bf = cp.tile([1, C], F32, name=f"{bname}_f")
        nc.sync.dma_start(bf[:, :], d[bname].ap().rearrange("(o c) -> o c", o=1))
        bcol[bname] = bf
    gamma_c = cp.tile([C, 1], F32)
    nc.sync.dma_start(gamma_c[:, :], d["gamma"].ap().rearrange("(c o) -> c o", o=1))
    beta_c = cp.tile([C, 1], F32)
    nc.sync.dma_start(beta_c[:, :], d["beta"].ap().rearrange("(c o) -> c o", o=1))

    # ---------------- x transpose to xT + stats ----------------
    # transposes use the deep pro psum ring; psum->sbuf copies alternate
    # DVE / ScalarE so neither engine gates the prologue; per-8-tile-group
    # stats (sum via reduce, sumsq via stt+accum) trail the copies on DVE.
    xT = big.tile([C, nq], BF16)
    ngr = 8
    s1 = cp.tile([C, ngr], F32)
    s2 = cp.tile([C, ngr], F32)
    xsq = big.tile([C, nq // ngr], BF16)  # overwritten per group; accum matters
    gw = nq // ngr
    for t in range(nj):
        pst = pro.tile([128, 128], F32, name="xtp", tag="pro")
        nc.tensor.transpose(pst[:, :], xsb[:, t, :], ident[:, :])
        sl = slice(t * 128, (t + 1) * 128)
        if t % 4 == 0:
            nc.vector.tensor_copy(xT[:, sl], pst[:, :])
        else:
            nc.scalar.copy(xT[:, sl], pst[:, :])
        if t % 4 == 3:
            g = t // 4
            gsl = slice(g * gw, (g + 1) * gw)
            nc.vector.reduce_sum(s1[:, g:g + 1], xT[:, gsl], axis=AX.X)
            nc.vector.scalar_tensor_tensor(
                xsq[:, :], xT[:, gsl], 1.0, xT[:, gsl],
                op0=ALU.mult, op1=ALU.mult, accum_out=s2[:, g:g + 1])

    pro.release()
    kpool = tc.alloc_tile_pool(name="kpool", bufs=4, space="PSUM")

    # bias transposes + c_p (emitted after the x transposes so the PE
    # stream is not gated on the late bias DMAs)
    bq_c = cp.tile([C, 1], F32)
    bk_c = cp.tile([C, 1], F32)
    bv_c = cp.tile([C, 1], BF16)
    bqs_row = cp.tile([1, C], F32)
    nc.vector.tensor_scalar_mul(bqs_row[:, :], bcol["bq"][:, :], scale ** 0.5)
    bks_row = cp.tile([1, C], F32)
    nc.vector.tensor_scalar_mul(bks_row[:, :], bcol["bk"][:, :], scale ** 0.5)
    for row, dst in ((bqs_row, bq_c), (bks_row, bk_c), (bcol["bv"], bv_c)):
        ps_b = misc.tile([C, 512], F32, name="ps_b", tag="misc")
        nc.tensor.transpose(ps_b[:, 0:1], row[:, :], ident[0:1, 0:1])
        nc.vector.tensor_copy(dst[:, :], ps_b[:, 0:1])

    # c_p = bv @ wp + bp  (rank-1 fold of the v/p biases)
    cp_row = cp.tile([1, C], BF16)
    ps_cp = misc.tile([C, 512], F32, name="ps_cp", tag="misc")
    nc.tensor.matmul(ps_cp[0:1, 0:C], bv_c[:, :], wsb["wp"][:, :],
                     start=True, stop=True)
    nc.vector.tensor_tensor(cp_row[:, :], ps_cp[0:1, 0:C], bcol["bp"][:, :],
                            op=ALU.add)

    def _flat_out(src_ap):
        yf = d["y"].ap().rearrange("n c -> (n c)").rearrange(
            "(p f) -> p f", p=128)
        nc.sync.dma_start(yf, src_ap)

    if stage == 1:
        xt_f = big.tile([C, nq], F32)
        nc.vector.tensor_copy(xt_f[:, :], xT[:, :])
        _flat_out(xt_f[:, :])
        for p in (sb_t, sb_p, pro, misc, big, cp):
            p.release()
        return

    st2 = cp.tile([C, 2], F32)
    nc.vector.reduce_sum(st2[:, 0:1], s1[:, :], axis=AX.X)
    nc.vector.reduce_sum(st2[:, 1:2], s2[:, :], axis=AX.X)
    gps = misc.tile([GROUPS, 512], F32, name="gps", tag="misc")
    # gmat is pre-scaled by 1/(nq*C/G): gps = [mean | E[x^2]] directly
    nc.tensor.matmul(gps[:, 0:2], gmat[:, :], st2[:, :], start=True, stop=True)
    gstat = cp.tile([GROUPS, 6], F32)
    nc.vector.tensor_copy(gstat[:, 0:2], gps[:, 0:2])
    nc.vector.tensor_mul(gstat[:, 2:3], gstat[:, 0:1], gstat[:, 0:1])     # mean^2
    nc.vector.tensor_sub(gstat[:, 3:4], gstat[:, 1:2], gstat[:, 2:3])     # var
    # rstd = exp(-0.5*ln(var+eps)) — ln/exp live in one ACT table set
    eps_c = cp.tile([GROUPS, 1], F32)
    nc.vector.memset(eps_c[:, :], EPS)
    nc.scalar.activation(gstat[:, 4:5], gstat[:, 3:4], AF.Ln, bias=eps_c[:, :])
    nc.scalar.activation(gstat[:, 5:6], gstat[:, 4:5], AF.Exp, scale=-0.5)
    pair = cp.tile([GROUPS, 2], F32)
    nc.vector.tensor_copy(pair[:, 0:1], gstat[:, 5:6])
    nc.vector.tensor_copy(pair[:, 1:2], gstat[:, 0:1])
    bcp = misc.tile([C, 512], F32, name="bcp", tag="misc")
    nc.tensor.matmul(bcp[:, 0:2], gtmat[:, :], pair[:, :], start=True, stop=True)
    ab = cp.tile([C, 2], F32)
    nc.vector.tensor_mul(ab[:, 0:1], gamma_c[:, :], bcp[:, 0:1])          # a
    nc.vector.tensor_mul(ab[:, 1:2], bcp[:, 1:2], ab[:, 0:1])             # mean*a
    nc.vector.tensor_sub(ab[:, 1:2], beta_c[:, :], ab[:, 1:2])            # b

    # xnT in 512-col pieces, each chased by its kT projection (PE + DVE);
    # only piece 0 runs in the prologue, the rest stream into the main loop
    xnT = big.tile([C, nq], BF16)
    kT = big.tile([C, nq], F8)

    def emit_kT(ch):
        sl = slice(ch * 512, (ch + 1) * 512)
        ps = kpool.tile([128, 512], F32, name="k_ps", tag="kpool")
        nc.tensor.matmul(ps[:, :], wsb["wk"][:, :], xnT[:, sl],
                         start=True, stop=True)
        if ch % 4 != 3:
            nc.scalar.activation(kT[:, sl], ps[:, :], AF.Identity,
                                 bias=bk_c[:, :])
        else:
            nc.vector.tensor_scalar_add(kT[:, sl], ps[:, :], bk_c[:, :])

    for ch in range(nch):
        sl = slice(ch * 512, (ch + 1) * 512)
        nc.vector.tensor_scalar(
            xnT[:, sl], xT[:, sl], ab[:, 0:1], ab[:, 1:2],
            op0=ALU.mult, op1=ALU.add)
    for ch in range(nch):
        emit_kT(ch)

    if stage == 2:
        xn_f = big.tile([C, nq], F32)
        nc.vector.tensor_copy(xn_f[:, :], xnT[:, :])
        _flat_out(xn_f[:, :])
        for p in (sb_t, sb_p, pro, misc, big, cp):
            p.release()
        return

    # ---------------- projections: v1 (deferred), qT (per chunk) --------
    v1 = big.tile([128, nj, 130], F8)
    nc.vector.memset(v1[:, :, 128:130], 1.0)

    def emit_v1(t, pool=None):
        ps = (pool or misc).tile([128, 512], F32, name="v_ps",
                                 tag="misc" if pool is None else "kpool")
        nc.tensor.matmul(ps[:, 0:128], xnT[:, t * 128:(t + 1) * 128],
                         wsb["wv"][:, :], start=True, stop=True)
        nc.vector.tensor_copy(v1[:, t, 0:128], ps[:, 0:128])

    qT = big.tile([C, nq], F8)

    def emit_qT(ch, pool=None):
        sl = slice(ch * 512, (ch + 1) * 512)
        ps = (pool or misc).tile([128, 512], F32, name="q_ps",
                                 tag="misc" if pool is None else "kpool")
        nc.tensor.matmul(ps[:, :], wsb["wq"][:, :], xnT[:, sl],
                         start=True, stop=True)
        if ch < 2:
            nc.scalar.activation(qT[:, sl], ps[:, :], AF.Identity,
                                 bias=bq_c[:, :])
        else:
            nc.vector.tensor_scalar_add(qT[:, sl], ps[:, :], bq_c[:, :])

    emit_qT(0, pool=kpool)
    emit_qT(1, pool=kpool)
    kpool.release()
    p_sc = tc.alloc_tile_pool(name="p_sc", bufs=2, space="PSUM")
    p_out = tc.alloc_tile_pool(name="p_out", bufs=1, space="PSUM")

    if stage == 3:
        for t in range(nj):
            emit_v1(t)
        kt_f = big.tile([C, nq], F32)
        nc.vector.tensor_copy(kt_f[:, :], kT[:, :])
        _flat_out(kt_f[:, :])
        for p in (sb_t, sb_p, p_out, p_sc, misc, big, cp):
            p.release()
        return

    # ---------------- main attention loop (software-pipelined) ----------
    y_r = d["y"].ap().rearrange("(c q p) ch -> c p q ch", q=qsn, p=128)
    LA = 4  # scores/exp run LA steps ahead of attnV

    state = {}  # per-chunk in-flight psum/sbuf handles

    def emit_scores(step):
        ch, jp = divmod(step, npair)
        sc = p_sc.tile([128, 2, 512], F32, name="sc")
        for jj in range(2):
            j = 2 * jp + jj
            nc.tensor.matmul(sc[:, jj, 0:chq],
                             kT[:, j * 128:(j + 1) * 128],
                             qT[:, ch * chq:(ch + 1) * chq],
                             start=True, stop=True)
        pT = sb_p.tile([128, 2, 512], F8, name="pT")
        # exp on ScalarE, shifted by -3.5 to fit fp8-e4m3 range (softmax
        # is shift-invariant)
        nc.scalar.activation(pT[:, :, 0:chq], sc[:, :, 0:chq], AF.Exp,
                             bias=negc[:, :])
        state[(ch, jp)] = pT

    def emit_attnv(step):
        ch, jp = divmod(step, npair)
        if jp == 0:
            out_ac = p_out.tile([128, 2, 512], F32, name="out_ac")
            state[ch] = (out_ac, {}, {})
        out_ac, first_mm, last_mm = state[ch]
        pT = state.pop((ch, jp))
        for b_ in range(2):
            for s in range(2):
                qs = 2 * b_ + s
                mm = nc.tensor.matmul(
                    out_ac[:, b_, 129 * s:129 * s + 129],
                    pT[:, 0:2, qs * 128:(qs + 1) * 128],
                    v1[:, 2 * jp:2 * jp + 2, 0:129],
                    start=(jp == 0 and s == 0),
                    stop=(jp == npair - 1 and s == 1),
                    perf_mode=mybir.MatmulPerfMode.DoubleRow)
                first_mm.setdefault((b_, s), mm)
                last_mm[(b_, s)] = mm
        if jp == npair - 1:
            for b_ in range(2):
                add_dep_helper(first_mm[(b_, 1)].ins, first_mm[(b_, 0)].ins,
                               sync=False, reason="psum group start order")
                add_dep_helper(last_mm[(b_, 1)].ins, last_mm[(b_, 0)].ins,
                               sync=False, reason="psum group stop order")

    tails = {}

    def emit_tail_dve(ch):
        out_ac, _, _ = state.pop(ch)
        rcp = sb_t.tile([128, 2, 2, 1], F32, name="rcp")
        den = out_ac[:, :, 128:128 + 258].rearrange(
            "p b (s x) -> p b s x", s=2, x=129)[:, :, :, 0:1]
        nc.vector.reciprocal(rcp[:, :, :, :], den)
        attn = sb_t.tile([128, qsn, 128], BF16, name="attn")
        ysb = sb_t.tile([128, qsn, 128], F32, name="ysb")
        for qs in range(qsn):
            b_, s = qs // 2, qs % 2
            nc.vector.tensor_scalar_mul(
                attn[:, qs, :], out_ac[:, b_, 129 * s:129 * s + 128],
                rcp[:, b_, s, :])
        if stage == 4:
            ysb4 = sb_t.tile([128, qsn, 128], F32, name="ysb4")
            nc.vector.tensor_copy(ysb4[:, :, :], attn[:, :, :])
            nc.sync.dma_start(y_r[ch], ysb4[:, :, :])
            return
        tails[ch] = (attn, ysb, {})

    def emit_tail_piece(ch, k):
        # piece k: transpose subtile k (k<qsn); project subtile k-1 (k>0)
        attn, ysb, atTs = tails[ch]
        if k < qsn:
            pst = misc.tile([128, 512], BF16, name="pst", tag="misc")
            nc.tensor.transpose(pst[:, 0:128], attn[:, k, :], ident_b[:, :])
            atT = sb_t.tile([128, 128], BF16, name="atT")
            nc.vector.tensor_copy(atT[:, :], pst[:, 0:128])
            atTs[k] = atT
        if k > 0:
            qs = k - 1
            psf = misc.tile([128, 512], F32, name="psf", tag="misc")
            nc.tensor.matmul(psf[:, 0:128], atTs.pop(qs)[:, :],
                             wsb["wp"][:, :], start=True, stop=False)
            nt = ch * chq + qs * 128
            nc.tensor.matmul(psf[:, 0:128], xnT[:, nt:nt + 128],
                             ident_b[:, :], start=False, stop=False)
            nc.tensor.matmul(psf[:, 0:128], ones_row[:, 0:128],
                             cp_row[:, :], start=False, stop=True)
            nc.vector.tensor_copy(ysb[:, qs, :], psf[:, 0:128])
            if qs == qsn - 1:
                nc.sync.dma_start(y_r[ch], ysb[:, :, :])
                del tails[ch]

    nsteps = nch * npair
    pend = {}
    for step in range(nsteps + LA + 2 * qsn + 3):
        if step < nsteps:
            ch_s, jp_s = divmod(step, npair)
            if jp_s == npair // 2 and ch_s + 2 < nch:
                emit_qT(ch_s + 2)
            emit_scores(step)
        if 2 * step < nj:
            emit_v1(2 * step)
            emit_v1(2 * step + 1)
        if LA <= step < nsteps + LA:
            a = step - LA
            emit_attnv(a)
            ch_a, jp_a = divmod(a, npair)
            if jp_a == npair - 1:
                emit_tail_dve(ch_a)
                if stage != 4:
                    for k in range(qsn + 1):
                        pend.setdefault(step + 1 + k, []).append((ch_a, k))
        for (c_, k_) in pend.pop(step, []):
            emit_tail_piece(c_, k_)

    for p in (sb_t, sb_p, p_out, p_sc, misc, big, cp):
        p.release()


def build_module(nq=NQ, stage=99):
    nc = bacc.Bacc("TRN2", target_bir_lowering=False, debug=False,
                   enable_asserts=False)
    d = {}
    d["x"] = nc.dram_tensor("x", [nq, C], F32, kind="ExternalInput")
    d["gamma"] = nc.dram_tensor("gamma", [C], F32, kind="ExternalInput")
    d["beta"] = nc.dram_tensor("beta", [C], F32, kind="ExternalInput")
    for wname in ("wq", "wk", "wv", "wp"):
        d[wname] = nc.dram_tensor(wname, [C, C], F32, kind="ExternalInput")
    for bname in ("bq", "bk", "bv", "bp"):
        d[bname] = nc.dram_tensor(bname, [C], F32, kind="ExternalInput")
    d["y"] = nc.dram_tensor("y", [nq, C], F32, kind="ExternalOutput")

    d["ident"] = nc.inline_tensor(np.eye(C, dtype=np.float32), "ident")
    gm = np.zeros((C, GROUPS), np.float32)
    gm[np.arange(C), np.arange(C) // (C // GROUPS)] = 1.0
    d["gtmat"] = nc.inline_tensor(np.ascontiguousarray(gm.T), "gtmat")
    d["gmat"] = nc.inline_tensor(gm / (nq * (C // GROUPS)), "gmat")

    with tile.TileContext(nc) as tc:
        _body(tc, d, nq, stage=stage)
    nc.compile()
    return nc


_CACHED_NC = None


def kernel(x, gamma, beta, wq, bq, wk, bk, wv, bv, wp, bp):
    global _CACHED_NC, LAST_RESULTS
    x = np.asarray(x, np.float32)
    assert x.shape == (B, H, W, C), x.shape
    if _CACHED_NC is None:
        _CACHED_NC = build_module(NQ)
    nc = _CACHED_NC

    shared = {
        "gamma": np.asarray(gamma, np.float32),
        "beta": np.asarray(beta, np.float32),
        "wq": np.asarray(wq, np.float32), "bq": np.asarray(bq, np.float32),
        "wk": np.asarray(wk, np.float32), "bk": np.asarray(bk, np.float32),
        "wv": np.asarray(wv, np.float32), "bv": np.asarray(bv, np.float32),
        "wp": np.asarray(wp, np.float32), "bp": np.asarray(bp, np.float32),
    }
    xf = x.reshape(B, NQ, C)
    in_maps = [dict(shared, x=np.ascontiguousarray(xf[b_])) for b_ in range(B)]
    res = run_bass_kernel_spmd(nc, in_maps, core_ids=list(range(N_CORES)))
    LAST_RESULTS = res
    out = np.stack([res.results[b_]["y"] for b_ in range(B)])
    return out.reshape(B, H, W, C).astype(np.float32)


# revision 31
# speedup vs baseline: 1.0211x; 1.0009x over previous
"""Trainium2 Bass kernel for GroupNorm + single-head self-attention block.

Computes, per batch element b (data-parallel over 8 NeuronCores):
    xn = group_norm(x[b])                 # 8 groups over (H, W, C/8)
    q, k, v = xn@wq+bq, xn@wk+bk, xn@wv+bv
    attn = softmax(q @ k.T / sqrt(C))
    y[b] = xn + (attn @ v) @ wp + bp

Shapes: x [8, 64, 64, 128] -> per core [4096, 128], C=128.

v2 dataflow (per core), all-bf16 compute path:
  - x DMA'd in 4 slabs; PE transposes -> xT f32; DVE copies accumulate
    group-norm stats per 8-tile group (sum via reduce, sumsq via
    scalar_tensor_tensor with accum_out).
  - xnT = a*xT + b emitted as bf16 (used for q/k/v and the residual).
  - qT/kT bf16 with bias folded into the DVE psum->sbuf copy; v tiles
    bf16 with a ones column; bv/bp folded into c_p = bv@wp + bp added
    via a rank-1 matmul in the output projection.
  - main loop is software-pipelined across chunk boundaries: score
    matmuls (and exp on ScalarE) run 2 steps ahead of the attn@V
    accumulation, so the PE stream never waits on the chunk tails.
  - out accumulation keeps the baseline trick: column 128 of each
    129-wide slot accumulates the softmax denominator for free.
  - chunk tail: normalize (DVE), PE transpose (bf16), project with wp +
    residual (identity matmul on xnT) + c_p rank-1, DVE copy, DMA out.
"""

import numpy as np

import concourse.bass as bass
import concourse.bacc as bacc
import concourse.mybir as mybir
import concourse.tile as tile
from concourse.bass_utils import run_bass_kernel_spmd
from concourse.tile import add_dep_helper

F32 = mybir.dt.float32
F32R = mybir.dt.float32r
BF16 = mybir.dt.bfloat16
F8 = mybir.dt.float8e4
U8 = mybir.dt.uint8
# fp8-e4m3 exp bits: bits = sat_u8(s*8*log2e + B); shift of -3.5 and the
# Schraudolph offset folded into B; negatives saturate to 0 (= fp8 +0)
EXP_A = 8 * 1.4426950408889634
EXP_B = -3.5 * 8 * 1.4426950408889634 + 56 - 0.25
AF = mybir.ActivationFunctionType
ALU = mybir.AluOpType
AX = mybir.AxisListType

B, H, W, C = 8, 64, 64, 128
NQ = H * W  # 4096 tokens per batch element
GROUPS = 8
EPS = 1e-5
N_CORES = 8

LAST_RESULTS = None  # BassKernelResults of the most recent run (for profiling)


def _body(tc, d, nq, stage=99):
    nc = tc.nc
    nj = nq // 128              # k-tiles (32)
    chq = min(512, nq)          # q-chunk width
    nch = nq // chq             # chunks (8)
    qsn = chq // 128            # q-subtiles per chunk (4)
    assert qsn == 4 and nj % 2 == 0, (nq, qsn)
    npair = nj // 2             # jp pairs per chunk (16)
    scale = float(C) ** -0.5

    cp = tc.alloc_tile_pool(name="consts", bufs=1)
    big = tc.alloc_tile_pool(name="big", bufs=1)
    misc = tc.alloc_tile_pool(name="misc", bufs=2, space="PSUM")
    pro = tc.alloc_tile_pool(name="pro", bufs=6, space="PSUM")
    p_sc = p_out = None  # created after pro.release() (PSUM is bank-budgeted)
    sb_p = tc.alloc_tile_pool(name="sb_p", bufs=4)
    sb_t = tc.alloc_tile_pool(name="sb_t", bufs=2)

    # ---------------- x DMA first (transposes gate the whole prologue) ----
    xsb = big.tile([128, nj, 128], F32)
    x_r = d["x"].ap().rearrange("(s t p) c -> s p t c", p=128, t=4)
    nc.sync.dma_start(xsb[:, 0:4, :], x_r[0])
    ident = cp.tile([C, C], F32)
    nc.sync.dma_start(ident[:, :], d["ident"].ap())
    for s in range(1, 8):
        nc.sync.dma_start(xsb[:, s * 4:(s + 1) * 4, :], x_r[s])
    # warm the Ln/Exp activation tables while the DMAs stream
    warm = cp.tile([1, 2], F32)
    nc.vector.memset(warm[:, :], 1.0)
    nc.scalar.activation(warm[:, 0:1], warm[:, 0:1], AF.Ln, bias=warm[:, 1:2])
    nc.scalar.activation(warm[:, 0:1], warm[:, 0:1], AF.Exp)
    ident_b = cp.tile([C, C], BF16)
    nc.vector.tensor_copy(ident_b[:, :], ident[:, :])
    gmat = cp.tile([C, GROUPS], F32)
    nc.sync.dma_start(gmat[:, :], d["gmat"].ap())
    gtmat = cp.tile([GROUPS, C], F32)
    nc.sync.dma_start(gtmat[:, :], d["gtmat"].ap())
    ones_row = cp.tile([1, C], BF16)
    nc.vector.memset(ones_row[:, :], 1.0)
    negc = cp.tile([128, 1], F32)
    nc.vector.memset(negc[:, :], -3.5)

    wsb = {}
    for wname in ("wq", "wk", "wv", "wp"):
        wf = cp.tile([C, C], F32, name=f"{wname}_f")
        nc.sync.dma_start(wf[:, :], d[wname].ap())
        wsb[wname] = cp.tile([C, C], BF16, name=f"{wname}_sb")
        if wname in ("wq", "wk"):  # fold sqrt of attention scale into each
            nc.vector.tensor_scalar_mul(wsb[wname][:, :], wf[:, :],
                                        scale ** 0.5)
        else:
            nc.vector.tensor_copy(wsb[wname][:, :], wf[:, :])
    # bias columns [C,1] f32 for per-partition folds (q/k); bv as bf16 col
    bcol = {}
    for bname in ("bq", "bk", "bv", "bp"):
        bf = cp.tile([1, C], F32, name=f"{bname}_f")
        nc.sync.dma_start(bf[:, :], d[bname].ap().rearrange("(o c) -> o c", o=1))
        bcol[bname] = bf
    gamma_c = cp.tile([C, 1], F32)
    nc.sync.dma_start(gamma_c[:, :], d["gamma"].ap().rearrange("(c o) -> c o", o=1))
    beta_c = cp.tile([C, 1], F32)
    nc.sync.dma_start(beta_c[:, :], d["beta"].ap().rearrange("(c o) -> c o", o=1))

    # ---------------- x transpose to xT + stats ----------------
    # transposes use the deep pro psum ring; psum->sbuf copies alternate
    # DVE / ScalarE so neither engine gates the prologue; per-8-tile-group
    # stats (sum via reduce, sumsq via stt+accum) trail the copies on DVE.
    xT = big.tile([C, nq], BF16)
    ngr = 8
    s1 = cp.tile([C, ngr], F32)
    s2 = cp.tile([C, ngr], F32)
    xsq = big.tile([C, nq // ngr], BF16)  # overwritten per group; accum matters
    gw = nq // ngr
    for t in range(nj):
        pst = pro.tile([128, 128], F32, name="xtp", tag="pro")
        nc.tensor.transpose(pst[:, :], xsb[:, t, :], ident[:, :])
        sl = slice(t * 128, (t + 1) * 128)
        if t % 4 == 0:
            nc.vector.tensor_copy(xT[:, sl], pst[:, :])
        else:
            nc.scalar.copy(xT[:, sl], pst[:, :])
        if t % 4 == 3:
            g = t // 4
            gsl = slice(g * gw, (g + 1) * gw)
            nc.vector.reduce_sum(s1[:, g:g + 1], xT[:, gsl], axis=AX.X)
            nc.vector.scalar_tensor_tensor(
                xsq[:, :], xT[:, gsl], 1.0, xT[:, gsl],
                op0=ALU.mult, op1=ALU.mult, accum_out=s2[:, g:g + 1])

    pro.release()
    kpool = tc.alloc_tile_pool(name="kpool", bufs=4, space="PSUM")

    # bias transposes + c_p (emitted after the x transposes so the PE
    # stream is not gated on the late bias DMAs)
    bq_c = cp.tile([C, 1], F32)
    bk_c = cp.tile([C, 1], F32)
    bv_c = cp.tile([C, 1], BF16)
    bqs_row = cp.tile([1, C], F32)
    nc.vector.tensor_scalar_mul(bqs_row[:, :], bcol["bq"][:, :], scale ** 0.5)
    bks_row = cp.tile([1, C], F32)
    nc.vector.tensor_scalar_mul(bks_row[:, :], bcol["bk"][:, :], scale ** 0.5)
    for row, dst in ((bqs_row, bq_c), (bks_row, bk_c), (bcol["bv"], bv_c)):
        ps_b = misc.tile([C, 512], F32, name="ps_b", tag="misc")
        nc.tensor.transpose(ps_b[:, 0:1], row[:, :], ident[0:1, 0:1])
        nc.vector.tensor_copy(dst[:, :], ps_b[:, 0:1])

    # c_p = bv @ wp + bp  (rank-1 fold of the v/p biases)
    cp_row = cp.tile([1, C], BF16)
    ps_cp = misc.tile([C, 512], F32, name="ps_cp", tag="misc")
    nc.tensor.matmul(ps_cp[0:1, 0:C], bv_c[:, :], wsb["wp"][:, :],
                     start=True, stop=True)
    nc.vector.tensor_tensor(cp_row[:, :], ps_cp[0:1, 0:C], bcol["bp"][:, :],
                            op=ALU.add)

    def _flat_out(src_ap):
        yf = d["y"].ap().rearrange("n c -> (n c)").rearrange(
            "(p f) -> p f", p=128)
        nc.sync.dma_start(yf, src_ap)

    if stage == 1:
        xt_f = big.tile([C, nq], F32)
        nc.vector.tensor_copy(xt_f[:, :], xT[:, :])
        _flat_out(xt_f[:, :])
        for p in (sb_t, sb_p, pro, misc, big, cp):
            p.release()
        return

    st2 = cp.tile([C, 2], F32)
    nc.vector.reduce_sum(st2[:, 0:1], s1[:, :], axis=AX.X)
    nc.vector.reduce_sum(st2[:, 1:2], s2[:, :], axis=AX.X)
    gps = misc.tile([GROUPS, 512], F32, name="gps", tag="misc")
    # gmat is pre-scaled by 1/(nq*C/G): gps = [mean | E[x^2]] directly
    nc.tensor.matmul(gps[:, 0:2], gmat[:, :], st2[:, :], start=True, stop=True)
    gstat = cp.tile([GROUPS, 6], F32)
    nc.vector.tensor_copy(gstat[:, 0:2], gps[:, 0:2])
    nc.vector.tensor_mul(gstat[:, 2:3], gstat[:, 0:1], gstat[:, 0:1])     # mean^2
    nc.vector.tensor_sub(gstat[:, 3:4], gstat[:, 1:2], gstat[:, 2:3])     # var
    # rstd = exp(-0.5*ln(var+eps)) — ln/exp live in one ACT table set
    eps_c = cp.tile([GROUPS, 1], F32)
    nc.vector.memset(eps_c[:, :], EPS)
    nc.scalar.activation(gstat[:, 4:5], gstat[:, 3:4], AF.Ln, bias=eps_c[:, :])
    nc.scalar.activation(gstat[:, 5:6], gstat[:, 4:5], AF.Exp, scale=-0.5)
    pair = cp.tile([GROUPS, 2], F32)
    nc.vector.tensor_copy(pair[:, 0:1], gstat[:, 5:6])
    nc.vector.tensor_copy(pair[:, 1:2], gstat[:, 0:1])
    bcp = misc.tile([C, 512], F32, name="bcp", tag="misc")
    nc.tensor.matmul(bcp[:, 0:2], gtmat[:, :], pair[:, :], start=True, stop=True)
    ab = cp.tile([C, 2], F32)
    nc.vector.tensor_mul(ab[:, 0:1], gamma_c[:, :], bcp[:, 0:1])          # a
    nc.vector.tensor_mul(ab[:, 1:2], bcp[:, 1:2], ab[:, 0:1])             # mean*a
    nc.vector.tensor_sub(ab[:, 1:2], beta_c[:, :], ab[:, 1:2])            # b

    # xnT in 512-col pieces, each chased by its kT projection (PE + DVE);
    # only piece 0 runs in the prologue, the rest stream into the main loop
    xnT = big.tile([C, nq], BF16)
    kT = big.tile([C, nq], F8)

    def emit_kT(ch):
        sl = slice(ch * 512, (ch + 1) * 512)
        ps = kpool.tile([128, 512], F32, name="k_ps", tag="kpool")
        nc.tensor.matmul(ps[:, :], wsb["wk"][:, :], xnT[:, sl],
                         start=True, stop=True)
        if ch % 4 != 3:
            nc.scalar.activation(kT[:, sl], ps[:, :], AF.Identity,
                                 bias=bk_c[:, :])
        else:
            nc.vector.tensor_scalar_add(kT[:, sl], ps[:, :], bk_c[:, :])

    for ch in range(nch):
        sl = slice(ch * 512, (ch + 1) * 512)
        nc.vector.tensor_scalar(
            xnT[:, sl], xT[:, sl], ab[:, 0:1], ab[:, 1:2],
            op0=ALU.mult, op1=ALU.add)
    for ch in range(nch):
        emit_kT(ch)

    if stage == 2:
        xn_f = big.tile([C, nq], F32)
        nc.vector.tensor_copy(xn_f[:, :], xnT[:, :])
        _flat_out(xn_f[:, :])
        for p in (sb_t, sb_p, pro, misc, big, cp):
            p.release()
        return

    # ---------------- projections: v1 (deferred), qT (per chunk) --------
    v1 = big.tile([128, nj, 130], F8)
    nc.vector.memset(v1[:, :, 128:130], 1.0)

    def emit_v1(t, pool=None):
        ps = (pool or misc).tile([128, 512], F32, name="v_ps",
                                 tag="misc" if pool is None else "kpool")
        nc.tensor.matmul(ps[:, 0:128], xnT[:, t * 128:(t + 1) * 128],
                         wsb["wv"][:, :], start=True, stop=True)
        nc.vector.tensor_copy(v1[:, t, 0:128], ps[:, 0:128])

    qT = big.tile([C, nq], F8)

    def emit_qT(ch, pool=None):
        sl = slice(ch * 512, (ch + 1) * 512)
        ps = (pool or misc).tile([128, 512], F32, name="q_ps",
                                 tag="misc" if pool is None else "kpool")
        nc.tensor.matmul(ps[:, :], wsb["wq"][:, :], xnT[:, sl],
                         start=True, stop=True)
        if ch < 2:
            nc.scalar.activation(qT[:, sl], ps[:, :], AF.Identity,
                                 bias=bq_c[:, :])
        else:
            nc.vector.tensor_scalar_add(qT[:, sl], ps[:, :], bq_c[:, :])

    emit_qT(0, pool=kpool)
    emit_qT(1, pool=kpool)
    kpool.release()
    p_sc = tc.alloc_tile_pool(name="p_sc", bufs=2, space="PSUM")
    p_out = tc.alloc_tile_pool(name="p_out", bufs=1, space="PSUM")

    if stage == 3:
        for t in range(nj):
            emit_v1(t)
        kt_f = big.tile([C, nq], F32)
        nc.vector.tensor_copy(kt_f[:, :], kT[:, :])
        _flat_out(kt_f[:, :])
        for p in (sb_t, sb_p, p_out, p_sc, misc, big, cp):
            p.release()
        return

    # ---------------- main attention loop (software-pipelined) ----------
    y_r = d["y"].ap().rearrange("(c q p) ch -> c p q ch", q=qsn, p=128)
    LA = 3  # scores/exp run LA steps ahead of attnV

    state = {}  # per-chunk in-flight psum/sbuf handles

    def emit_scores(step):
        ch, jp = divmod(step, npair)
        sc = p_sc.tile([128, 2, 512], F32, name="sc")
        for jj in range(2):
            j = 2 * jp + jj
            nc.tensor.matmul(sc[:, jj, 0:chq],
                             kT[:, j * 128:(j + 1) * 128],
                             qT[:, ch * chq:(ch + 1) * chq],
                             start=True, stop=True)
        pT = sb_p.tile([128, 2, 512], F8, name="pT")
        # exp on ScalarE, shifted by -3.5 to fit fp8-e4m3 range (softmax
        # is shift-invariant)
        nc.scalar.activation(pT[:, :, 0:chq], sc[:, :, 0:chq], AF.Exp,
                             bias=negc[:, :])
        state[(ch, jp)] = pT

    def emit_attnv(step):
        ch, jp = divmod(step, npair)
        if jp == 0:
            out_ac = p_out.tile([128, 2, 512], F32, name="out_ac")
            state[ch] = (out_ac, {}, {})
        out_ac, first_mm, last_mm = state[ch]
        pT = state.pop((ch, jp))
        for b_ in range(2):
            for s in range(2):
                qs = 2 * b_ + s
                mm = nc.tensor.matmul(
                    out_ac[:, b_, 129 * s:129 * s + 129],
                    pT[:, 0:2, qs * 128:(qs + 1) * 128],
                    v1[:, 2 * jp:2 * jp + 2, 0:129],
                    start=(jp == 0 and s == 0),
                    stop=(jp == npair - 1 and s == 1),
                    perf_mode=mybir.MatmulPerfMode.DoubleRow)
                first_mm.setdefault((b_, s), mm)
                last_mm[(b_, s)] = mm
        if jp == npair - 1:
            for b_ in range(2):
                add_dep_helper(first_mm[(b_, 1)].ins, first_mm[(b_, 0)].ins,
                               sync=False, reason="psum group start order")
                add_dep_helper(last_mm[(b_, 1)].ins, last_mm[(b_, 0)].ins,
                               sync=False, reason="psum group stop order")

    tails = {}

    def emit_tail_dve(ch):
        out_ac, _, _ = state.pop(ch)
        rcp = sb_t.tile([128, 2, 2, 1], F32, name="rcp")
        den = out_ac[:, :, 128:128 + 258].rearrange(
            "p b (s x) -> p b s x", s=2, x=129)[:, :, :, 0:1]
        nc.vector.reciprocal(rcp[:, :, :, :], den)
        attn = sb_t.tile([128, qsn, 128], BF16, name="attn")
        ysb = sb_t.tile([128, qsn, 128], F32, name="ysb")
        for qs in range(qsn):
            b_, s = qs // 2, qs % 2
            nc.vector.tensor_scalar_mul(
                attn[:, qs, :], out_ac[:, b_, 129 * s:129 * s + 128],
                rcp[:, b_, s, :])
        if stage == 4:
            ysb4 = sb_t.tile([128, qsn, 128], F32, name="ysb4")
            nc.vector.tensor_copy(ysb4[:, :, :], attn[:, :, :])
            nc.sync.dma_start(y_r[ch], ysb4[:, :, :])
            return
        tails[ch] = (attn, ysb, {})

    def emit_tail_piece(ch, k):
        # piece k: transpose subtile k (k<qsn); project subtile k-1 (k>0)
        attn, ysb, atTs = tails[ch]
        if k < qsn:
            pst = misc.tile([128, 512], BF16, name="pst", tag="misc")
            nc.tensor.transpose(pst[:, 0:128], attn[:, k, :], ident_b[:, :])
            atT = sb_t.tile([128, 128], BF16, name="atT")
            nc.vector.tensor_copy(atT[:, :], pst[:, 0:128])
            atTs[k] = atT
        if k > 0:
            qs = k - 1
            psf = misc.tile([128, 512], F32, name="psf", tag="misc")
            nc.tensor.matmul(psf[:, 0:128], atTs.pop(qs)[:, :],
                             wsb["wp"][:, :], start=True, stop=False)
            nt = ch * chq + qs * 128
            nc.tensor.matmul(psf[:, 0:128], xnT[:, nt:nt + 128],
                             ident_b[:, :], start=False, stop=False)
            nc.tensor.matmul(psf[:, 0:128], ones_row[:, 0:128],
                             cp_row[:, :], start=False, stop=True)
            nc.vector.tensor_copy(ysb[:, qs, :], psf[:, 0:128])
            if qs == qsn - 1:
                nc.sync.dma_start(y_r[ch], ysb[:, :, :])
                del tails[ch]

    nsteps = nch * npair
    pend = {}
    for step in range(nsteps + LA + 2 * qsn + 3):
        if step < nsteps:
            ch_s, jp_s = divmod(step, npair)
            if jp_s == npair // 2 and ch_s + 2 < nch:
                emit_qT(ch_s + 2)
            emit_scores(step)
        if 2 * step < nj:
            emit_v1(2 * step)
            emit_v1(2 * step + 1)
        if LA <= step < nsteps + LA:
            a = step - LA
            emit_attnv(a)
            ch_a, jp_a = divmod(a, npair)
            if jp_a == npair - 1:
                emit_tail_dve(ch_a)
                if stage != 4:
                    for k in range(qsn + 1):
                        pend.setdefault(step + 1 + k, []).append((ch_a, k))
        for (c_, k_) in pend.pop(step, []):
            emit_tail_piece(c_, k_)

    for p in (sb_t, sb_p, p_out, p_sc, misc, big, cp):
        p.release()


def build_module(nq=NQ, stage=99):
    nc = bacc.Bacc("TRN2", target_bir_lowering=False, debug=False,
                   enable_asserts=False)
    d = {}
    d["x"] = nc.dram_tensor("x", [nq, C], F32, kind="ExternalInput")
    d["gamma"] = nc.dram_tensor("gamma", [C], F32, kind="ExternalInput")
    d["beta"] = nc.dram_tensor("beta", [C], F32, kind="ExternalInput")
    for wname in ("wq", "wk", "wv", "wp"):
        d[wname] = nc.dram_tensor(wname, [C, C], F32, kind="ExternalInput")
    for bname in ("bq", "bk", "bv", "bp"):
        d[bname] = nc.dram_tensor(bname, [C], F32, kind="ExternalInput")
    d["y"] = nc.dram_tensor("y", [nq, C], F32, kind="ExternalOutput")

    d["ident"] = nc.inline_tensor(np.eye(C, dtype=np.float32), "ident")
    gm = np.zeros((C, GROUPS), np.float32)
    gm[np.arange(C), np.arange(C) // (C // GROUPS)] = 1.0
    d["gtmat"] = nc.inline_tensor(np.ascontiguousarray(gm.T), "gtmat")
    d["gmat"] = nc.inline_tensor(gm / (nq * (C // GROUPS)), "gmat")

    with tile.TileContext(nc) as tc:
        _body(tc, d, nq, stage=stage)
    nc.compile()
    return nc


_CACHED_NC = None


def kernel(x, gamma, beta, wq, bq, wk, bk, wv, bv, wp, bp):
    global _CACHED_NC, LAST_RESULTS
    x = np.asarray(x, np.float32)
    assert x.shape == (B, H, W, C), x.shape
    if _CACHED_NC is None:
        _CACHED_NC = build_module(NQ)
    nc = _CACHED_NC

    shared = {
        "gamma": np.asarray(gamma, np.float32),
        "beta": np.asarray(beta, np.float32),
        "wq": np.asarray(wq, np.float32), "bq": np.asarray(bq, np.float32),
        "wk": np.asarray(wk, np.float32), "bk": np.asarray(bk, np.float32),
        "wv": np.asarray(wv, np.float32), "bv": np.asarray(bv, np.float32),
        "wp": np.asarray(wp, np.float32), "bp": np.asarray(bp, np.float32),
    }
    xf = x.reshape(B, NQ, C)
    in_maps = [dict(shared, x=np.ascontiguousarray(xf[b_])) for b_ in range(B)]
    res = run_bass_kernel_spmd(nc, in_maps, core_ids=list(range(N_CORES)))
    LAST_RESULTS = res
    out = np.stack([res.results[b_]["y"] for b_ in range(B)])
    return out.reshape(B, H, W, C).astype(np.float32)
